# revision 6
# baseline (speedup 1.0000x reference)
"""Trainium2 Bass kernel for nn_DynamicDASBlock.

out = x + einsum('boc,bchw->bohw', einsum('be,eoc->boc', softmax(MLP(scores)), expert_w), x)
data-parallel over B across 8 NeuronCores (2 samples per core).

Two key tricks:
1. Residual fold: softmax weights sum to 1, so
   x + (sum_e r_e E_e) @ x == (sum_e r_e (E_e + I)) @ x; the host adds I to each
   (transposed) expert matrix once and the device does a single GEMM.
2. Compensated fp32r GEMM (MODE "f32r3"): fp32 matmul runs at 4 cycles/row on
   the PE while fp32r (tf32-like rounded operands) runs at 1 cycle/row.
   Splitting W = Wr + Wl and x = xr + xl (rounded halves; the subtraction is
   exact) gives W@x ~= Wr@xr + Wr@xl + Wl@xr with fp32-level accuracy at 3
   cycles/row. The dropped Wl@xl term is O(2^-22) relative.
"""

import sys
from contextlib import ExitStack

import numpy as np

_TRN_REPO = "/opt/trn_rl_repo"
if _TRN_REPO not in sys.path:
    sys.path.insert(0, _TRN_REPO)

B, C, H, W = 16, 256, 128, 128
E, D, HID = 3, 3, 16
HWP = H * W            # 16384 spatial positions
NCORES = 8
BLOC = B // NCORES     # 2 samples per core
P = 128                # partitions
KCH = C // P           # 2 row/contraction chunks
MMW = 512              # matmul free dim (one PSUM bank, fp32)

MODE = "f32r3"         # "fp32" | "f32r3" | "f32r1"
NW = 2048              # spatial slice width per DMA tile
NSL = HWP // NW        # slices per sample
NSUB = NW // MMW       # matmul groups per slice

_CACHE = {}


def _body(tc, bass, mybir, x_d, ew_d, st_d, f1w_d, f1b_d, f2w_d, f2b_d, sel_d, out_d):
    f32 = mybir.dt.float32
    f32r = mybir.dt.float32r
    AF = mybir.ActivationFunctionType
    AX = mybir.AxisListType
    nc = tc.nc
    with ExitStack() as ctx:
        const = ctx.enter_context(tc.tile_pool(name="const", bufs=1))
        xpool = ctx.enter_context(tc.tile_pool(name="xin", bufs=4))
        opool = ctx.enter_context(tc.tile_pool(name="oout", bufs=4))
        psum = ctx.enter_context(tc.tile_pool(name="psum", bufs=6, space="PSUM"))
        if MODE == "f32r3":
            xrpool = ctx.enter_context(tc.tile_pool(name="xr", bufs=4))
            xlpool = ctx.enter_context(tc.tile_pool(name="xl", bufs=4))

        # ---- load constants ----
        # expert weights, transposed (+I): ew_t[e][p, k*C+o] = expert_w[o, k*128+p] (+I)
        ew_t = []
        for e in range(E):
            t = const.tile([P, KCH * C], f32, name=f"ew{e}", tag=f"ew{e}")
            nc.sync.dma_start(
                t[:].rearrange("p (k o) -> p k o", k=KCH),
                ew_d.ap()[e].rearrange("(k p) o -> p k o", p=P),
            )
            ew_t.append(t)

        st_t = const.tile([D, BLOC], f32, name="st", tag="st")
        nc.sync.dma_start(st_t[:], st_d.ap())
        f1w_t = const.tile([D, HID], f32, name="f1w", tag="f1w")
        nc.sync.dma_start(f1w_t[:], f1w_d.ap())
        f1b_t = const.tile([HID, 1], f32, name="f1b", tag="f1b")
        nc.sync.dma_start(f1b_t[:], f1b_d.ap())
        f2w_t = const.tile([HID, E], f32, name="f2w", tag="f2w")
        nc.sync.dma_start(f2w_t[:], f2w_d.ap())
        f2b_t = const.tile([BLOC, E], f32, name="f2b", tag="f2b")
        nc.sync.dma_start(f2b_t[:], f2b_d.ap())

        # per-local-sample one-hot selector rows for the broadcast matmul
        sel_t = []
        for b in range(BLOC):
            s = const.tile([BLOC, P], f32, name=f"sel{b}", tag=f"sel{b}")
            nc.sync.dma_start(s[:], sel_d.ap()[b])
            sel_t.append(s)

        # ---- routing MLP (B on the free axis, all samples of this core) ----
        h_ps = psum.tile([HID, BLOC], f32, name="h_ps", tag="small", bufs=2)
        nc.tensor.matmul(h_ps[:], f1w_t[:], st_t[:])
        hT = const.tile([HID, BLOC], f32, name="hT", tag="hT")
        nc.scalar.activation(hT[:], h_ps[:], AF.Relu, bias=f1b_t[:, 0:1], scale=1.0)

        lg_ps = psum.tile([BLOC, E], f32, name="lg_ps", tag="small", bufs=2)
        nc.tensor.matmul(lg_ps[:], hT[:], f2w_t[:])
        lg = const.tile([BLOC, E], f32, name="lg", tag="lg")
        nc.vector.tensor_add(lg[:], lg_ps[:], f2b_t[:])

        # softmax along free axis (E=3)
        mx = const.tile([BLOC, 1], f32, name="mx", tag="mx")
        nc.vector.reduce_max(mx[:], lg[:], axis=AX.X)
        nmx = const.tile([BLOC, 1], f32, name="nmx", tag="nmx")
        nc.vector.tensor_scalar_mul(nmx[:], mx[:], -1.0)
        exps = const.tile([BLOC, E], f32, name="exps", tag="exps")
        nc.scalar.activation(exps[:], lg[:], AF.Exp, bias=nmx[:, 0:1], scale=1.0)
        sm = const.tile([BLOC, 1], f32, name="sm", tag="sm")
        nc.vector.reduce_sum(sm[:], exps[:], axis=AX.X)
        rcp = const.tile([BLOC, 1], f32, name="rcp", tag="rcp")
        nc.vector.reciprocal(rcp[:], sm[:])
        r_t = const.tile([BLOC, E], f32, name="r_t", tag="r_t")
        nc.vector.tensor_scalar_mul(r_t[:], exps[:], rcp[:, 0:1])

        # ---- per-sample dynamic weight synthesis ----
        wb_t, wr_t, wl_t = [], [], []
        for b in range(BLOC):
            rb_ps = psum.tile([P, E], f32, name=f"rb_ps{b}", tag="small", bufs=2)
            nc.tensor.matmul(rb_ps[:], sel_t[b][:], r_t[:])
            rb = const.tile([P, E], f32, name=f"rb{b}", tag=f"rb{b}")
            nc.vector.tensor_copy(rb[:], rb_ps[:])

            wb = const.tile([P, KCH * C], f32, name=f"wb{b}", tag=f"wb{b}")
            tmp = const.tile([P, KCH * C], f32, name=f"wtmp{b}", tag=f"wtmp{b}")
            nc.vector.tensor_scalar_mul(wb[:], ew_t[0][:], rb[:, 0:1])
            nc.vector.tensor_scalar_mul(tmp[:], ew_t[1][:], rb[:, 1:2])
            nc.vector.tensor_add(wb[:], wb[:], tmp[:])
            nc.vector.tensor_scalar_mul(tmp[:], ew_t[2][:], rb[:, 2:3])
            nc.vector.tensor_add(wb[:], wb[:], tmp[:])
            wb_t.append(wb)

            if MODE in ("f32r3", "f32r1"):
                wr = const.tile([P, KCH * C], f32r, name=f"wr{b}", tag=f"wr{b}")
                nc.vector.tensor_copy(wr[:], wb[:])
                wr_t.append(wr)
            if MODE == "f32r3":
                wl = const.tile([P, KCH * C], f32r, name=f"wl{b}", tag=f"wl{b}")
                nc.vector.tensor_sub(wl[:], wb[:], wr[:].bitcast(f32))
                wl_t.append(wl)

        # ---- main GEMM: out[b, o, n] = sum_c w'[o, c] x[b, c, n] ----
        for b in range(BLOC):
            for s in range(NSL):
                xk, xrk, xlk = [], [], []
                for k in range(KCH):
                    if MODE == "f32r1":
                        xt = xpool.tile([P, NW], f32r, name=f"x{b}_{s}_{k}", tag="x")
                    else:
                        xt = xpool.tile([P, NW], f32, name=f"x{b}_{s}_{k}", tag="x")
                    nc.sync.dma_start(
                        xt[:], x_d.ap()[b, k * P : (k + 1) * P, s * NW : (s + 1) * NW]
                    )
                    xk.append(xt)
                    if MODE == "f32r3":
                        xr = xrpool.tile([P, NW], f32r, name=f"xr{b}_{s}_{k}", tag="xr")
                        nc.scalar.copy(xr[:], xt[:])
                        xrk.append(xr)
                        xl = xlpool.tile([P, NW], f32r, name=f"xl{b}_{s}_{k}", tag="xl")
                        nc.vector.tensor_sub(xl[:], xt[:], xr[:].bitcast(f32))
                        xlk.append(xl)
                for m in range(KCH):
                    ot = opool.tile([P, NW], f32, name=f"o{b}_{s}_{m}", tag="o")
                    for j in range(NSUB):
                        ps = psum.tile([P, MMW], f32, name=f"mm{b}_{s}_{m}_{j}", tag="mm")
                        js = slice(j * MMW, (j + 1) * MMW)
                        if MODE == "fp32":
                            mms = [
                                (wb_t[b], xk[k], k) for k in range(KCH)
                            ]
                        elif MODE == "f32r1":
                            mms = [(wr_t[b], xk[k], k) for k in range(KCH)]
                        else:
                            mms = []
                            for k in range(KCH):
                                mms.append((wr_t[b], xrk[k], k))
                                mms.append((wr_t[b], xlk[k], k))
                                mms.append((wl_t[b], xrk[k], k))
                        for i, (wt, xt_, k) in enumerate(mms):
                            nc.tensor.matmul(
                                ps[:],
                                wt[:, k * C + m * P : k * C + m * P + P],
                                xt_[:, js],
                                start=(i == 0),
                                stop=(i == len(mms) - 1),
                            )
                        if (m * NSUB + j) % 2 == 0:
                            nc.vector.tensor_copy(ot[:, js], ps[:])
                        else:
                            nc.scalar.copy(ot[:, js], ps[:])
                    nc.sync.dma_start(
                        out_d.ap()[b, m * P : (m + 1) * P, s * NW : (s + 1) * NW], ot[:]
                    )


def _build(reps=1):
    import concourse.bacc as bacc
    import concourse.bass as bass
    import concourse.tile as tile
    from concourse import mybir

    f32 = mybir.dt.float32
    f32r = mybir.dt.float32r
    xdt = f32r if MODE == "f32r1" else f32
    nc = bacc.Bacc("TRN2", target_bir_lowering=False, debug=False, num_devices=NCORES)
    x_d = nc.dram_tensor("x", [BLOC, C, HWP], xdt, kind="ExternalInput")
    ew_d = nc.dram_tensor("ew", [E, C, C], f32, kind="ExternalInput")
    st_d = nc.dram_tensor("scoresT", [D, BLOC], f32, kind="ExternalInput")
    f1w_d = nc.dram_tensor("fc1_w", [D, HID], f32, kind="ExternalInput")
    f1b_d = nc.dram_tensor("fc1_b", [HID, 1], f32, kind="ExternalInput")
    f2w_d = nc.dram_tensor("fc2_w", [HID, E], f32, kind="ExternalInput")
    f2b_d = nc.dram_tensor("fc2_b_rep", [BLOC, E], f32, kind="ExternalInput")
    sel_d = nc.dram_tensor("sel", [BLOC, BLOC, P], f32, kind="ExternalInput")
    out_d = nc.dram_tensor("out", [BLOC, C, HWP], f32, kind="ExternalOutput")
    with tile.TileContext(nc) as tc:
        for _ in range(reps):
            _body(
                tc, bass, mybir, x_d, ew_d, st_d, f1w_d, f1b_d, f2w_d, f2b_d, sel_d,
                out_d,
            )
    nc.compile()
    return nc


def _get_nc(reps=1):
    key = ("nc", MODE, reps)
    if key not in _CACHE:
        _CACHE[key] = _build(reps)
    return _CACHE[key]


def _round_tf32(a):
    return (a.view(np.uint32) & np.uint32(0xFFFFE000)).view(np.float32)


def make_in_maps(inputs):
    """Shard FULL inputs into 8 per-core input maps (host-side layout prep only)."""
    x = np.ascontiguousarray(np.asarray(inputs["x"], dtype=np.float32))
    scores = np.asarray(inputs["scores"], dtype=np.float32)
    fc1_w = np.ascontiguousarray(np.asarray(inputs["fc1_w"], dtype=np.float32))
    fc1_b = np.asarray(inputs["fc1_b"], dtype=np.float32)
    fc2_w = np.ascontiguousarray(np.asarray(inputs["fc2_w"], dtype=np.float32))
    fc2_b = np.asarray(inputs["fc2_b"], dtype=np.float32)
    expert_w = np.asarray(inputs["expert_w"], dtype=np.float32)

    # transpose experts to [e, c_in, c_out] and fold in the residual identity
    ew = np.ascontiguousarray(expert_w.transpose(0, 2, 1))
    idx = np.arange(C)
    ew[:, idx, idx] += np.float32(1.0)

    x_r = x.reshape(B, C, HWP)
    if MODE == "f32r1":
        x_r = _round_tf32(x_r)
    f1b = np.ascontiguousarray(fc1_b.reshape(HID, 1))
    f2b = np.ascontiguousarray(np.tile(fc2_b.reshape(1, E), (BLOC, 1)))
    sel = np.zeros((BLOC, BLOC, P), dtype=np.float32)
    for b in range(BLOC):
        sel[b, b, :] = 1.0

    in_maps = []
    for c in range(NCORES):
        g0 = c * BLOC
        in_maps.append(
            {
                "x": x_r[g0 : g0 + BLOC],
                "ew": ew,
                "scoresT": np.ascontiguousarray(scores[g0 : g0 + BLOC].T),
                "fc1_w": fc1_w,
                "fc1_b": f1b,
                "fc2_w": fc2_w,
                "fc2_b_rep": f2b,
                "sel": sel,
            }
        )
    return in_maps


def run_spmd(inputs, trace=False):
    """Run the Bass kernel on cores 0-7. Returns BassKernelResults."""
    from concourse import bass_utils

    nc = _get_nc()
    in_maps = make_in_maps(inputs)
    return bass_utils.run_bass_kernel_spmd(
        nc, in_maps, core_ids=list(range(NCORES)), trace=trace
    )


def kernel(**inputs) -> np.ndarray:
    res = run_spmd(inputs, trace=False)
    out = np.stack([r["out"] for r in res.results], axis=0)  # [8, BLOC, C, HWP]
    return out.reshape(B, C, H, W)
